# revision 18
# baseline (speedup 1.0000x reference)
"""Trainium2 Bass kernel for DistillLossSimpleMSE (segment_reduce).

Math (per object o, with uniform segments of P points):
    x   = net_out[o*P:(o+1)*P]                [P, D]
    m   = mask_pts[o]                         [M, P] in {0,1}
    e   = nan_to_num(mask_embs[o*M:(o+1)*M])  [M, D]
    sum_sq = sum_m [ sum_p m*||x_p||^2 + cnt_m*||e_m||^2 - 2 e_m . (sum_p m x_p) ]
    out = sum_sq / (D * total_points)

Sharding: object-parallel, 1 object per core (8 objects, 8 cores).

Device kernel per core accumulates in PSUM over all P points (bf16 matmuls,
f32 PSUM accumulate):
    acc[32, 384] = m^T.T @ [x | x*x | 1]
      cols   0:128 -> mx[m, d],  cols 128:256 -> sum_p m x^2,  col 256 -> cnt
Host does the tiny per-mask finale with the embeddings.

Layout trick for full DMA bandwidth: a straight contiguous [128, 4096] tile of
x (16 KB/partition descriptors) has partition p holding rows 32p..32p+31, so
its column-slice [:, c*128:(c+1)*128] is exactly the [point, d] matmul operand
for the stride-32 point class {base + 32p + c}. The mask (contiguous [128,
16384] int32 view) is transposed on-chip through the PE with a stride-32 free
AP so its chunks cover the same point classes.

The rhs buffer holds three contiguous 4096-col bf16 regions [x | x^2 | ones];
the matmul rhs AP gathers one 128-col slice from each region with a regular
two-level access pattern (stride 4096), so x only needs one fast contiguous
convert (DVE 2x mode) and one contiguous square — no strided interleave
copies. The ones region makes out cols 256:384 all equal cnt.

Multi-wait instructions are legalized via bass_rust.generate_event_semaphores
(TRN2 allows only one semaphore wait per compute instruction).
"""

import os

import numpy as np
import ml_dtypes

import bass_rust
import concourse.bass as bass
import concourse.mybir as mybir
import concourse.tile as tile
from concourse.bass_utils import run_bass_kernel_spmd

N_CORES = 8
N_OBJ, P, M, D = 8, 65536, 32, 128

VIEW_P = 128                 # mask flat view partitions
VIEW_F = M * P // VIEW_P     # 16384 view cols; view[r, f] = mask[r//4, (r%4)*16384 + f]
BLK = 4096                   # view cols per block (= points per x-tile)
NBLK = VIEW_F // BLK         # 4 mask blocks
NCLS = BLK // 128            # 32 stride-32 point classes per block
NT = 16                      # x tiles of [128, 4096]
OUTC = 3 * D                 # 384 output cols: [mx | m@x^2 | cnt*128]
NXB = 3                      # x-tile landing buffers (f32)
NRHS = 3                     # rhs buffers [x | x^2 | ones] bf16

F32 = mybir.dt.float32
BF16 = mybir.dt.bfloat16
I32 = mybir.dt.int32

LAST = None      # BassKernelResults of the most recent run (for test harness)
_NC_CACHE = {}


def _build_nc():
    nc = bass.Bass()
    x = nc.dram_tensor("x", [P, D], F32, kind="ExternalInput")
    mask = nc.dram_tensor("mask", [VIEW_P, VIEW_F], I32, kind="ExternalInput")
    out = nc.dram_tensor("out", [M, OUTC], F32, kind="ExternalOutput")

    # x tile view: [16 tiles, 128 partitions, 32*128 contiguous]
    xt = x[:, :].rearrange("(j p s) d -> j p (s d)", p=128, s=32)

    with tile.TileContext(nc) as tc:
        with (
            tc.tile_pool(name="singles", bufs=1) as singles,
            tc.tile_pool(name="psingles", bufs=1, space="PSUM") as psingles,
        ):
            # Persistent tiles only: pool-reallocated tiles go through Tile's
            # release machinery whose extra waits collide with the PE 1-wait
            # codegen limit more often.
            ident_const = nc.inline_tensor(
                np.eye(128, dtype=np.float32).astype(ml_dtypes.bfloat16),
                name="identc",
            )
            ident = singles.tile([128, 128], BF16, tag="ident")
            nc.sync.dma_start(out=ident, in_=ident_const[:, :])

            # rhs buffers: three contiguous 4096-col regions [x | x^2 | ones].
            rhs_bufs = []
            for j in range(NRHS):
                rb = singles.tile(
                    [128, 3 * BLK], BF16, name=f"rhsbuf{j}", tag=f"rhsbuf{j}"
                )
                nc.vector.memset(rb[:, 2 * BLK:3 * BLK], 1.0)
                rhs_bufs.append(rb)

            xb_bufs = [
                singles.tile([128, BLK], F32, name=f"xb{j}", tag=f"xb{j}")
                for j in range(NXB)
            ]
            mi_bufs = [
                singles.tile([VIEW_P, BLK], I32, name=f"mi{j}", tag=f"mi{j}")
                for j in range(2)
            ]
            mf_bufs = [
                singles.tile([VIEW_P, BLK], BF16, name=f"mf{j}", tag=f"mf{j}")
                for j in range(2)
            ]
            mt_bufs = [
                singles.tile([VIEW_P, BLK], BF16, name=f"mt{j}", tag=f"mt{j}")
                for j in range(2)
            ]
            ps4_bufs = [
                psingles.tile([128, 4, 128], BF16, name=f"ps4{j}", tag=f"ps4{j}")
                for j in range(2)
            ]
            acc = psingles.tile([M, OUTC], F32, tag="acc")

            n_mm = NBLK * 4 * NCLS
            k = 0
            jx = 0
            for b in range(NBLK):
                mi = mi_bufs[b % 2]
                mf = mf_bufs[b % 2]
                # Piecewise DMA + convert so the first transposes can start
                # ~4us after the block's mask DMA begins instead of ~14us.
                # DVE, not GpSimd: the GpSimd CAST is ~4x slower and its
                # SBUF port contention starves concurrent DVE ops.
                for piece in range(4):
                    lo, hi = piece * (BLK // 4), (piece + 1) * (BLK // 4)
                    nc.sync.dma_start(
                        out=mi[:, lo:hi], in_=mask[:, b * BLK + lo:b * BLK + hi]
                    )
                    nc.vector.tensor_copy(mf[:, lo:hi], mi[:, lo:hi])

                # Transpose the mask block through the PE with stride-32 free
                # APs: transpose c yields, for every quarter q, the lhsT mask
                # columns of point class {q*16384 + b*4096 + 32p + c}.
                mt = mt_bufs[b % 2]
                # f' = 32p + c: class c picks free elems {32p + c}, stride 32
                mfv = mf.rearrange("r (p c) -> r c p", c=NCLS)  # [128, 32, 128]
                for h in range(NCLS // 4):
                    ps4 = ps4_bufs[h % 2]
                    for tt in range(4):
                        c = h * 4 + tt
                        nc.tensor.transpose(ps4[:, tt, :], mfv[:, c, :], ident)
                    nc.scalar.copy(
                        mt[:, h * 512:(h + 1) * 512],
                        ps4.rearrange("p t d -> p (t d)"),
                    )
                mtv = mt.rearrange("p (c m q) -> p c q m", c=NCLS, m=M, q=4)

                for q in range(4):
                    j = q * 4 + b          # x tile covering this block+quarter
                    xb = xb_bufs[jx % NXB]
                    rhs = rhs_bufs[jx % NRHS]
                    jx += 1
                    nc.sync.dma_start(out=xb, in_=xt[j, :, :])
                    # contiguous f32->bf16 convert on ACT (DVE CAST is 1x)
                    nc.scalar.copy(rhs[:, 0:BLK], xb)
                    # contiguous bf16 square on DVE (2x tensor_tensor mode)
                    nc.vector.tensor_mul(
                        rhs[:, BLK:2 * BLK], rhs[:, 0:BLK], rhs[:, 0:BLK]
                    )
                    rhs3 = rhs.rearrange("p (g u) -> p g u", g=3)
                    for c in range(NCLS):
                        nc.tensor.matmul(
                            acc[:, :],
                            lhsT=mtv[:, c, q, :],
                            rhs=rhs3[:, :, c * D:(c + 1) * D],
                            start=(k == 0),
                            stop=(k == n_mm - 1),
                        )
                        k += 1

            outs = singles.tile([M, OUTC], F32, tag="outs")
            nc.vector.tensor_copy(outs, acc)
            nc.sync.dma_start(out=out[:, :], in_=outs)
    # Split multi-wait instructions into EventSemaphore + instruction to
    # satisfy the TRN2 1-wait-per-instruction codegen limit.
    bass_rust.generate_event_semaphores(nc)
    return nc


def _get_nc():
    if "nc" not in _NC_CACHE:
        _NC_CACHE["nc"] = _build_nc()
    return _NC_CACHE["nc"]


def kernel(net_out, pt_offset, mask_embs, mask_pts, logit_scale):
    global LAST
    net_out = np.ascontiguousarray(np.asarray(net_out, dtype=np.float32))
    mask_pts = np.ascontiguousarray(np.asarray(mask_pts, dtype=np.int32))
    mask_embs = np.asarray(mask_embs, dtype=np.float32)

    nc = _get_nc()
    in_maps = [
        {
            "x": net_out[o * P:(o + 1) * P],
            "mask": mask_pts[o].reshape(VIEW_P, VIEW_F),
        }
        for o in range(N_CORES)
    ]
    trace = os.environ.get("KBENCH_TRACE", "0") == "1"
    res = run_bass_kernel_spmd(nc, in_maps, list(range(N_CORES)), trace=trace)
    LAST = res

    accs = np.stack([np.asarray(res.results[o]["out"]) for o in range(N_CORES)])
    mx = accs[:, :, 0:D].astype(np.float64)        # [8, 32, 128]
    sx2 = accs[:, :, D:2 * D].astype(np.float64)   # [8, 32, 128]
    cnt = accs[:, :, 2 * D].astype(np.float64)     # [8, 32]

    emb = np.nan_to_num(
        mask_embs.reshape(N_OBJ, M, D).astype(np.float64),
        nan=0.0, posinf=0.0, neginf=0.0,
    )
    t1 = sx2.sum(-1)
    t2 = cnt * (emb * emb).sum(-1)
    t3 = 2.0 * (emb * mx).sum(-1)
    sum_sq = (t1 + t2 - t3).sum()
    total = cnt.sum()
    val = sum_sq / (D * total) if total > 0 else 0.0
    return np.float32(val)


# revision 19
# speedup vs baseline: 1.0314x; 1.0314x over previous
"""Trainium2 Bass kernel for DistillLossSimpleMSE (segment_reduce).

Math (per object o, with uniform segments of P points):
    x   = net_out[o*P:(o+1)*P]                [P, D]
    m   = mask_pts[o]                         [M, P] in {0,1}
    e   = nan_to_num(mask_embs[o*M:(o+1)*M])  [M, D]
    sum_sq = sum_m [ sum_p m*||x_p||^2 + cnt_m*||e_m||^2 - 2 e_m . (sum_p m x_p) ]
    out = sum_sq / (D * total_points)

Sharding: object-parallel, 1 object per core (8 objects, 8 cores).

Device kernel per core accumulates in PSUM over all P points (bf16 matmuls,
f32 PSUM accumulate):
    acc[32, 384] = m^T.T @ [x | x*x | 1]
      cols   0:128 -> mx[m, d],  cols 128:256 -> sum_p m x^2,  col 256 -> cnt
Host does the tiny per-mask finale with the embeddings.

Layout trick for full DMA bandwidth: a straight contiguous [128, 4096] tile of
x (16 KB/partition descriptors) has partition p holding rows 32p..32p+31, so
its column-slice [:, c*128:(c+1)*128] is exactly the [point, d] matmul operand
for the stride-32 point class {base + 32p + c}. The mask (contiguous [128,
16384] int32 view) is transposed on-chip through the PE with a stride-32 free
AP so its chunks cover the same point classes.

The rhs buffer holds three contiguous 4096-col bf16 regions [x | x^2 | ones];
the matmul rhs AP gathers one 128-col slice from each region with a regular
two-level access pattern (stride 4096), so x only needs one fast contiguous
convert (DVE 2x mode) and one contiguous square — no strided interleave
copies. The ones region makes out cols 256:384 all equal cnt.

Multi-wait instructions are legalized via bass_rust.generate_event_semaphores
(TRN2 allows only one semaphore wait per compute instruction).
"""

import os

import numpy as np
import ml_dtypes

import bass_rust
import concourse.bass as bass
import concourse.mybir as mybir
import concourse.tile as tile
from concourse.bass_utils import run_bass_kernel_spmd

N_CORES = 8
N_OBJ, P, M, D = 8, 65536, 32, 128

VIEW_P = 128                 # mask flat view partitions
VIEW_F = M * P // VIEW_P     # 16384 view cols; view[r, f] = mask[r//4, (r%4)*16384 + f]
BLK = 4096                   # view cols per block (= points per x-tile)
NBLK = VIEW_F // BLK         # 4 mask blocks
NCLS = BLK // 128            # 32 stride-32 point classes per block
NT = 16                      # x tiles of [128, 4096]
OUTC = 3 * D                 # 384 output cols: [mx | m@x^2 | cnt*128]
NXB = 3                      # x-tile landing buffers (f32)
NRHS = 3                     # rhs buffers [x | x^2 | ones] bf16

F32 = mybir.dt.float32
BF16 = mybir.dt.bfloat16
I32 = mybir.dt.int32

LAST = None      # BassKernelResults of the most recent run (for test harness)
_NC_CACHE = {}


def _build_nc():
    nc = bass.Bass()
    x = nc.dram_tensor("x", [P, D], F32, kind="ExternalInput")
    mask = nc.dram_tensor("mask", [VIEW_P, VIEW_F], I32, kind="ExternalInput")
    out = nc.dram_tensor("out", [M, OUTC], F32, kind="ExternalOutput")

    # x tile view: [16 tiles, 128 partitions, 32*128 contiguous]
    xt = x[:, :].rearrange("(j p s) d -> j p (s d)", p=128, s=32)

    with tile.TileContext(nc) as tc:
        with (
            tc.tile_pool(name="singles", bufs=1) as singles,
            tc.tile_pool(name="psingles", bufs=1, space="PSUM") as psingles,
        ):
            # Persistent tiles only: pool-reallocated tiles go through Tile's
            # release machinery whose extra waits collide with the PE 1-wait
            # codegen limit more often.
            ident_const = nc.inline_tensor(
                np.eye(128, dtype=np.float32).astype(ml_dtypes.bfloat16),
                name="identc",
            )
            ident = singles.tile([128, 128], BF16, tag="ident")
            nc.sync.dma_start(out=ident, in_=ident_const[:, :])

            # rhs buffers: three contiguous 4096-col regions [x | x^2 | ones].
            rhs_bufs = []
            for j in range(NRHS):
                rb = singles.tile(
                    [128, 3 * BLK], BF16, name=f"rhsbuf{j}", tag=f"rhsbuf{j}"
                )
                nc.vector.memset(rb[:, 2 * BLK:3 * BLK], 1.0)
                rhs_bufs.append(rb)

            xb_bufs = [
                singles.tile([128, BLK], F32, name=f"xb{j}", tag=f"xb{j}")
                for j in range(NXB)
            ]
            MIP = BLK // 4
            mi_bufs = [
                singles.tile([VIEW_P, MIP], I32, name=f"mi{j}", tag=f"mi{j}")
                for j in range(3)
            ]
            mf_bufs = [
                singles.tile([VIEW_P, BLK], BF16, name=f"mf{j}", tag=f"mf{j}")
                for j in range(2)
            ]
            # One mt per block (written once, never reused) so the whole mask
            # pipeline can run ahead of the matmul stream.
            mt_bufs = [
                singles.tile([VIEW_P, BLK], BF16, name=f"mt{j}", tag=f"mt{j}")
                for j in range(NBLK)
            ]
            ps4_bufs = [
                psingles.tile([128, 4, 128], BF16, name=f"ps4{j}", tag=f"ps4{j}")
                for j in range(2)
            ]
            acc = psingles.tile([M, OUTC], F32, tag="acc")

            n_mm = NBLK * 4 * NCLS
            k = 0
            jx = 0
            for b in range(NBLK):
                mf = mf_bufs[b % 2]
                # Piecewise DMA + convert through small rotating int32 pieces.
                # DVE, not GpSimd: the GpSimd CAST is ~4x slower and its
                # SBUF port contention starves concurrent DVE ops.
                for piece in range(4):
                    mi = mi_bufs[(b * 4 + piece) % 3]
                    lo, hi = piece * MIP, (piece + 1) * MIP
                    nc.sync.dma_start(
                        out=mi, in_=mask[:, b * BLK + lo:b * BLK + hi]
                    )
                    nc.vector.tensor_copy(mf[:, lo:hi], mi)

                # Transpose the mask block through the PE with stride-32 free
                # APs: transpose c yields, for every quarter q, the lhsT mask
                # columns of point class {q*16384 + b*4096 + 32p + c}.
                mt = mt_bufs[b]
                # f' = 32p + c: class c picks free elems {32p + c}, stride 32
                mfv = mf.rearrange("r (p c) -> r c p", c=NCLS)  # [128, 32, 128]
                for h in range(NCLS // 4):
                    ps4 = ps4_bufs[h % 2]
                    for tt in range(4):
                        c = h * 4 + tt
                        nc.tensor.transpose(ps4[:, tt, :], mfv[:, c, :], ident)
                    nc.scalar.copy(
                        mt[:, h * 512:(h + 1) * 512],
                        ps4.rearrange("p t d -> p (t d)"),
                    )
                mtv = mt.rearrange("p (c m q) -> p c q m", c=NCLS, m=M, q=4)

                for q in range(4):
                    j = q * 4 + b          # x tile covering this block+quarter
                    xb = xb_bufs[jx % NXB]
                    rhs = rhs_bufs[jx % NRHS]
                    jx += 1
                    # Halved x chain (DMA -> ACT convert -> DVE square) to
                    # shorten the per-tile latency before matmuls can start.
                    for h2 in range(2):
                        lo, hi = h2 * (BLK // 2), (h2 + 1) * (BLK // 2)
                        nc.sync.dma_start(
                            out=xb[:, lo:hi], in_=xt[j, :, lo:hi]
                        )
                        # contiguous f32->bf16 convert on ACT (DVE CAST is 1x)
                        nc.scalar.copy(rhs[:, lo:hi], xb[:, lo:hi])
                        # contiguous bf16 square on DVE (2x tensor_tensor)
                        nc.vector.tensor_mul(
                            rhs[:, BLK + lo:BLK + hi],
                            rhs[:, lo:hi], rhs[:, lo:hi],
                        )
                    rhs3 = rhs.rearrange("p (g u) -> p g u", g=3)
                    for c in range(NCLS):
                        nc.tensor.matmul(
                            acc[:, :],
                            lhsT=mtv[:, c, q, :],
                            rhs=rhs3[:, :, c * D:(c + 1) * D],
                            start=(k == 0),
                            stop=(k == n_mm - 1),
                        )
                        k += 1

            outs = singles.tile([M, OUTC], F32, tag="outs")
            nc.vector.tensor_copy(outs, acc)
            nc.sync.dma_start(out=out[:, :], in_=outs)
    # Split multi-wait instructions into EventSemaphore + instruction to
    # satisfy the TRN2 1-wait-per-instruction codegen limit.
    bass_rust.generate_event_semaphores(nc)
    return nc


def _get_nc():
    if "nc" not in _NC_CACHE:
        _NC_CACHE["nc"] = _build_nc()
    return _NC_CACHE["nc"]


def kernel(net_out, pt_offset, mask_embs, mask_pts, logit_scale):
    global LAST
    net_out = np.ascontiguousarray(np.asarray(net_out, dtype=np.float32))
    mask_pts = np.ascontiguousarray(np.asarray(mask_pts, dtype=np.int32))
    mask_embs = np.asarray(mask_embs, dtype=np.float32)

    nc = _get_nc()
    in_maps = [
        {
            "x": net_out[o * P:(o + 1) * P],
            "mask": mask_pts[o].reshape(VIEW_P, VIEW_F),
        }
        for o in range(N_CORES)
    ]
    trace = os.environ.get("KBENCH_TRACE", "0") == "1"
    res = run_bass_kernel_spmd(nc, in_maps, list(range(N_CORES)), trace=trace)
    LAST = res

    accs = np.stack([np.asarray(res.results[o]["out"]) for o in range(N_CORES)])
    mx = accs[:, :, 0:D].astype(np.float64)        # [8, 32, 128]
    sx2 = accs[:, :, D:2 * D].astype(np.float64)   # [8, 32, 128]
    cnt = accs[:, :, 2 * D].astype(np.float64)     # [8, 32]

    emb = np.nan_to_num(
        mask_embs.reshape(N_OBJ, M, D).astype(np.float64),
        nan=0.0, posinf=0.0, neginf=0.0,
    )
    t1 = sx2.sum(-1)
    t2 = cnt * (emb * emb).sum(-1)
    t3 = 2.0 * (emb * mx).sum(-1)
    sum_sq = (t1 + t2 - t3).sum()
    total = cnt.sum()
    val = sum_sq / (D * total) if total > 0 else 0.0
    return np.float32(val)


# revision 20
# speedup vs baseline: 1.0544x; 1.0223x over previous
"""Trainium2 Bass kernel for DistillLossSimpleMSE (segment_reduce).

Math (per object o, with uniform segments of P points):
    x   = net_out[o*P:(o+1)*P]                [P, D]
    m   = mask_pts[o]                         [M, P] in {0,1}
    e   = nan_to_num(mask_embs[o*M:(o+1)*M])  [M, D]
    sum_sq = sum_m [ sum_p m*||x_p||^2 + cnt_m*||e_m||^2 - 2 e_m . (sum_p m x_p) ]
    out = sum_sq / (D * total_points)

Sharding: object-parallel, 1 object per core (8 objects, 8 cores).

Device kernel per core accumulates in PSUM over all P points (bf16 matmuls,
f32 PSUM accumulate):
    acc[32, 384] = m^T.T @ [x | x*x | 1]
      cols   0:128 -> mx[m, d],  cols 128:256 -> sum_p m x^2,  col 256 -> cnt
Host does the tiny per-mask finale with the embeddings.

Layout trick for full DMA bandwidth: a straight contiguous [128, 4096] tile of
x (16 KB/partition descriptors) has partition p holding rows 32p..32p+31, so
its column-slice [:, c*128:(c+1)*128] is exactly the [point, d] matmul operand
for the stride-32 point class {base + 32p + c}. The mask (contiguous [128,
16384] int32 view) is transposed on-chip through the PE with a stride-32 free
AP so its chunks cover the same point classes.

The rhs buffer holds three contiguous 4096-col bf16 regions [x | x^2 | ones];
the matmul rhs AP gathers one 128-col slice from each region with a regular
two-level access pattern (stride 4096), so x only needs one fast contiguous
convert (DVE 2x mode) and one contiguous square — no strided interleave
copies. The ones region makes out cols 256:384 all equal cnt.

Multi-wait instructions are legalized via bass_rust.generate_event_semaphores
(TRN2 allows only one semaphore wait per compute instruction).
"""

import os

import numpy as np
import ml_dtypes

import bass_rust
import concourse.bass as bass
import concourse.mybir as mybir
import concourse.tile as tile
from concourse.bass_utils import run_bass_kernel_spmd

N_CORES = 8
N_OBJ, P, M, D = 8, 65536, 32, 128

VIEW_P = 128                 # mask flat view partitions
VIEW_F = M * P // VIEW_P     # 16384 view cols; view[r, f] = mask[r//4, (r%4)*16384 + f]
BLK = 4096                   # view cols per block (= points per x-tile)
NBLK = VIEW_F // BLK         # 4 mask blocks
NCLS = BLK // 128            # 32 stride-32 point classes per block
NT = 16                      # x tiles of [128, 4096]
OUTC = 2 * D                 # 256 output cols: [mx | m@x^2]; cnt via convert accum
NXB = 3                      # x-tile landing buffers (f32)
NRHS = 3                     # rhs buffers [x | x^2 | ones] bf16

F32 = mybir.dt.float32
BF16 = mybir.dt.bfloat16
I32 = mybir.dt.int32

LAST = None      # BassKernelResults of the most recent run (for test harness)
_NC_CACHE = {}


def _build_nc():
    nc = bass.Bass()
    x = nc.dram_tensor("x", [P, D], F32, kind="ExternalInput")
    mask = nc.dram_tensor("mask", [VIEW_P, VIEW_F], I32, kind="ExternalInput")
    out = nc.dram_tensor("out", [M, OUTC], F32, kind="ExternalOutput")
    # per-(mask,quarter)-row, per-piece partial point counts (see host finale)
    cnts = nc.dram_tensor("cnts", [VIEW_P, NBLK * 4], F32, kind="ExternalOutput")

    # x tile view: [16 tiles, 128 partitions, 32*128 contiguous]
    xt = x[:, :].rearrange("(j p s) d -> j p (s d)", p=128, s=32)

    with tile.TileContext(nc) as tc:
        with (
            tc.tile_pool(name="singles", bufs=1) as singles,
            tc.tile_pool(name="psingles", bufs=1, space="PSUM") as psingles,
        ):
            # Persistent tiles only: pool-reallocated tiles go through Tile's
            # release machinery whose extra waits collide with the PE 1-wait
            # codegen limit more often.
            ident_const = nc.inline_tensor(
                np.eye(128, dtype=np.float32).astype(ml_dtypes.bfloat16),
                name="identc",
            )
            ident = singles.tile([128, 128], BF16, tag="ident")
            nc.sync.dma_start(out=ident, in_=ident_const[:, :])

            # rhs buffers: two contiguous 4096-col regions [x | x^2].
            rhs_bufs = [
                singles.tile(
                    [128, 2 * BLK], BF16, name=f"rhsbuf{j}", tag=f"rhsbuf{j}"
                )
                for j in range(NRHS)
            ]
            cnt_sb = singles.tile([VIEW_P, NBLK * 4], F32, tag="cnt_sb")

            xb_bufs = [
                singles.tile([128, BLK], F32, name=f"xb{j}", tag=f"xb{j}")
                for j in range(NXB)
            ]
            MIP = BLK // 4
            mi_bufs = [
                singles.tile([VIEW_P, MIP], I32, name=f"mi{j}", tag=f"mi{j}")
                for j in range(3)
            ]
            mf_bufs = [
                singles.tile([VIEW_P, BLK], BF16, name=f"mf{j}", tag=f"mf{j}")
                for j in range(2)
            ]
            # One mt per block (written once, never reused) so the whole mask
            # pipeline can run ahead of the matmul stream.
            mt_bufs = [
                singles.tile([VIEW_P, BLK], BF16, name=f"mt{j}", tag=f"mt{j}")
                for j in range(NBLK)
            ]
            ps4_bufs = [
                psingles.tile([128, 4, 128], BF16, name=f"ps4{j}", tag=f"ps4{j}")
                for j in range(2)
            ]
            acc = psingles.tile([M, OUTC], F32, tag="acc")

            n_mm = NBLK * 4 * NCLS
            k = 0
            jx = 0
            for b in range(NBLK):
                mf = mf_bufs[b % 2]
                # Piecewise DMA + convert through small rotating int32 pieces.
                # DVE, not GpSimd: the GpSimd CAST is ~4x slower and its
                # SBUF port contention starves concurrent DVE ops.
                for piece in range(4):
                    mi = mi_bufs[(b * 4 + piece) % 3]
                    lo, hi = piece * MIP, (piece + 1) * MIP
                    nc.sync.dma_start(
                        out=mi, in_=mask[:, b * BLK + lo:b * BLK + hi]
                    )
                    # convert + free-dim count in one DVE op
                    nc.vector.scalar_tensor_tensor(
                        out=mf[:, lo:hi],
                        in0=mi, scalar=0.0, in1=mi,
                        op0=mybir.AluOpType.add,
                        op1=mybir.AluOpType.bypass,
                        accum_out=cnt_sb[:, b * 4 + piece:b * 4 + piece + 1],
                    )

                # Transpose the mask block through the PE with stride-32 free
                # APs: transpose c yields, for every quarter q, the lhsT mask
                # columns of point class {q*16384 + b*4096 + 32p + c}.
                mt = mt_bufs[b]
                # f' = 32p + c: class c picks free elems {32p + c}, stride 32
                mfv = mf.rearrange("r (p c) -> r c p", c=NCLS)  # [128, 32, 128]
                for h in range(NCLS // 4):
                    ps4 = ps4_bufs[h % 2]
                    for tt in range(4):
                        c = h * 4 + tt
                        nc.tensor.transpose(ps4[:, tt, :], mfv[:, c, :], ident)
                    nc.scalar.copy(
                        mt[:, h * 512:(h + 1) * 512],
                        ps4.rearrange("p t d -> p (t d)"),
                    )
                mtv = mt.rearrange("p (c m q) -> p c q m", c=NCLS, m=M, q=4)

                for q in range(4):
                    j = q * 4 + b          # x tile covering this block+quarter
                    xb = xb_bufs[jx % NXB]
                    rhs = rhs_bufs[jx % NRHS]
                    jx += 1
                    # Halved x chain (DMA -> ACT convert -> DVE square) to
                    # shorten the per-tile latency before matmuls can start.
                    for h2 in range(2):
                        lo, hi = h2 * (BLK // 2), (h2 + 1) * (BLK // 2)
                        nc.sync.dma_start(
                            out=xb[:, lo:hi], in_=xt[j, :, lo:hi]
                        )
                        # contiguous f32->bf16 convert on ACT (DVE CAST is 1x)
                        nc.scalar.copy(rhs[:, lo:hi], xb[:, lo:hi])
                        # contiguous bf16 square on DVE (2x tensor_tensor)
                        nc.vector.tensor_mul(
                            rhs[:, BLK + lo:BLK + hi],
                            rhs[:, lo:hi], rhs[:, lo:hi],
                        )
                    rhs3 = rhs.rearrange("p (g u) -> p g u", g=2)
                    for c in range(NCLS):
                        nc.tensor.matmul(
                            acc[:, :],
                            lhsT=mtv[:, c, q, :],
                            rhs=rhs3[:, :, c * D:(c + 1) * D],
                            start=(k == 0),
                            stop=(k == n_mm - 1),
                        )
                        k += 1

            outs = singles.tile([M, OUTC], F32, tag="outs")
            nc.vector.tensor_copy(outs, acc)
            nc.sync.dma_start(out=out[:, :], in_=outs)
            nc.sync.dma_start(out=cnts[:, :], in_=cnt_sb)
    # Split multi-wait instructions into EventSemaphore + instruction to
    # satisfy the TRN2 1-wait-per-instruction codegen limit.
    bass_rust.generate_event_semaphores(nc)
    return nc


def _get_nc():
    if "nc" not in _NC_CACHE:
        _NC_CACHE["nc"] = _build_nc()
    return _NC_CACHE["nc"]


def kernel(net_out, pt_offset, mask_embs, mask_pts, logit_scale):
    global LAST
    net_out = np.ascontiguousarray(np.asarray(net_out, dtype=np.float32))
    mask_pts = np.ascontiguousarray(np.asarray(mask_pts, dtype=np.int32))
    mask_embs = np.asarray(mask_embs, dtype=np.float32)

    nc = _get_nc()
    in_maps = [
        {
            "x": net_out[o * P:(o + 1) * P],
            "mask": mask_pts[o].reshape(VIEW_P, VIEW_F),
        }
        for o in range(N_CORES)
    ]
    trace = os.environ.get("KBENCH_TRACE", "0") == "1"
    res = run_bass_kernel_spmd(nc, in_maps, list(range(N_CORES)), trace=trace)
    LAST = res

    accs = np.stack([np.asarray(res.results[o]["out"]) for o in range(N_CORES)])
    mx = accs[:, :, 0:D].astype(np.float64)        # [8, 32, 128]
    sx2 = accs[:, :, D:2 * D].astype(np.float64)   # [8, 32, 128]
    # cnt[m] = sum over quarters q and pieces of the per-row partials
    cnts = np.stack([np.asarray(res.results[o]["cnts"]) for o in range(N_CORES)])
    cnt = cnts.sum(-1).reshape(N_CORES, M, 4).sum(-1)  # [8, 32]

    emb = np.nan_to_num(
        mask_embs.reshape(N_OBJ, M, D).astype(np.float64),
        nan=0.0, posinf=0.0, neginf=0.0,
    )
    t1 = sx2.sum(-1)
    t2 = cnt * (emb * emb).sum(-1)
    t3 = 2.0 * (emb * mx).sum(-1)
    sum_sq = (t1 + t2 - t3).sum()
    total = cnt.sum()
    val = sum_sq / (D * total) if total > 0 else 0.0
    return np.float32(val)


# revision 21
# speedup vs baseline: 1.0705x; 1.0152x over previous
"""Trainium2 Bass kernel for DistillLossSimpleMSE (segment_reduce).

Math (per object o, with uniform segments of P points):
    x   = net_out[o*P:(o+1)*P]                [P, D]
    m   = mask_pts[o]                         [M, P] in {0,1}
    e   = nan_to_num(mask_embs[o*M:(o+1)*M])  [M, D]
    sum_sq = sum_m [ sum_p m*||x_p||^2 + cnt_m*||e_m||^2 - 2 e_m . (sum_p m x_p) ]
    out = sum_sq / (D * total_points)

Sharding: object-parallel, 1 object per core (8 objects, 8 cores).

Device kernel per core accumulates in PSUM over all P points (bf16 matmuls,
f32 PSUM accumulate):
    acc[32, 384] = m^T.T @ [x | x*x | 1]
      cols   0:128 -> mx[m, d],  cols 128:256 -> sum_p m x^2,  col 256 -> cnt
Host does the tiny per-mask finale with the embeddings.

Layout trick for full DMA bandwidth: a straight contiguous [128, 4096] tile of
x (16 KB/partition descriptors) has partition p holding rows 32p..32p+31, so
its column-slice [:, c*128:(c+1)*128] is exactly the [point, d] matmul operand
for the stride-32 point class {base + 32p + c}. The mask (contiguous [128,
16384] int32 view) is transposed on-chip through the PE with a stride-32 free
AP so its chunks cover the same point classes.

The rhs buffer holds three contiguous 4096-col bf16 regions [x | x^2 | ones];
the matmul rhs AP gathers one 128-col slice from each region with a regular
two-level access pattern (stride 4096), so x only needs one fast contiguous
convert (DVE 2x mode) and one contiguous square — no strided interleave
copies. The ones region makes out cols 256:384 all equal cnt.

Multi-wait instructions are legalized via bass_rust.generate_event_semaphores
(TRN2 allows only one semaphore wait per compute instruction).
"""

import os

import numpy as np
import ml_dtypes

import bass_rust
import concourse.bass as bass
import concourse.mybir as mybir
import concourse.tile as tile
from concourse.bass_utils import run_bass_kernel_spmd

N_CORES = 8
N_OBJ, P, M, D = 8, 65536, 32, 128

VIEW_P = 128                 # mask flat view partitions
VIEW_F = M * P // VIEW_P     # 16384 view cols; view[r, f] = mask[r//4, (r%4)*16384 + f]
BLK = 4096                   # view cols per block (= points per x-tile)
NBLK = VIEW_F // BLK         # 4 mask blocks
NCLS = BLK // 128            # 32 stride-32 point classes per block
NT = 16                      # x tiles of [128, 4096]
OUTC = 2 * D                 # 256 output cols: [mx | m@x^2]; cnt via convert accum
NXB = 3                      # x-tile landing buffers (f32)
NRHS = 3                     # rhs buffers [x | x^2 | ones] bf16

F32 = mybir.dt.float32
BF16 = mybir.dt.bfloat16
I32 = mybir.dt.int32

LAST = None      # BassKernelResults of the most recent run (for test harness)
_NC_CACHE = {}


def _build_nc():
    nc = bass.Bass()
    x = nc.dram_tensor("x", [P, D], F32, kind="ExternalInput")
    mask = nc.dram_tensor("mask", [VIEW_P, VIEW_F], I32, kind="ExternalInput")
    out = nc.dram_tensor("out", [M, OUTC], F32, kind="ExternalOutput")
    # per-(mask,quarter)-row, per-piece partial point counts (see host finale)
    cnts = nc.dram_tensor("cnts", [VIEW_P, NBLK * 4], F32, kind="ExternalOutput")

    # x tile view: [16 tiles, 128 partitions, 32*128 contiguous]
    xt = x[:, :].rearrange("(j p s) d -> j p (s d)", p=128, s=32)

    with tile.TileContext(nc) as tc:
        with (
            tc.tile_pool(name="singles", bufs=1) as singles,
            tc.tile_pool(name="psingles", bufs=1, space="PSUM") as psingles,
        ):
            # Persistent tiles only: pool-reallocated tiles go through Tile's
            # release machinery whose extra waits collide with the PE 1-wait
            # codegen limit more often.
            ident_const = nc.inline_tensor(
                np.eye(128, dtype=np.float32).astype(ml_dtypes.bfloat16),
                name="identc",
            )
            ident = singles.tile([128, 128], BF16, tag="ident")
            nc.sync.dma_start(out=ident, in_=ident_const[:, :])

            # rhs buffers: two contiguous 4096-col regions [x | x^2].
            rhs_bufs = [
                singles.tile(
                    [128, 2 * BLK], BF16, name=f"rhsbuf{j}", tag=f"rhsbuf{j}"
                )
                for j in range(NRHS)
            ]
            cnt_sb = singles.tile([VIEW_P, NBLK * 4], F32, tag="cnt_sb")

            xb_bufs = [
                singles.tile([128, BLK], F32, name=f"xb{j}", tag=f"xb{j}")
                for j in range(NXB)
            ]
            MIP = BLK // 4
            mi_bufs = [
                singles.tile([VIEW_P, MIP], I32, name=f"mi{j}", tag=f"mi{j}")
                for j in range(3)
            ]
            mf_bufs = [
                singles.tile([VIEW_P, BLK], BF16, name=f"mf{j}", tag=f"mf{j}")
                for j in range(2)
            ]
            # One mt per block (written once, never reused) so the whole mask
            # pipeline can run ahead of the matmul stream.
            mt_bufs = [
                singles.tile([VIEW_P, BLK], BF16, name=f"mt{j}", tag=f"mt{j}")
                for j in range(NBLK)
            ]
            ps4_bufs = [
                psingles.tile([128, 4, 128], BF16, name=f"ps4{j}", tag=f"ps4{j}")
                for j in range(2)
            ]
            acc = psingles.tile([M, OUTC], F32, tag="acc")

            n_mm = NBLK * 4 * NCLS

            def mask_chain(b):
                mf = mf_bufs[b % 2]
                # Piecewise DMA + convert through small rotating int32 pieces.
                # DVE, not GpSimd: the GpSimd CAST is ~4x slower and its
                # SBUF port contention starves concurrent DVE ops.
                for piece in range(4):
                    mi = mi_bufs[(b * 4 + piece) % 3]
                    lo, hi = piece * MIP, (piece + 1) * MIP
                    nc.sync.dma_start(
                        out=mi, in_=mask[:, b * BLK + lo:b * BLK + hi]
                    )
                    # convert + free-dim count in one DVE op
                    nc.vector.scalar_tensor_tensor(
                        out=mf[:, lo:hi],
                        in0=mi, scalar=0.0, in1=mi,
                        op0=mybir.AluOpType.add,
                        op1=mybir.AluOpType.bypass,
                        accum_out=cnt_sb[:, b * 4 + piece:b * 4 + piece + 1],
                    )

            def transposes(b, h0, h1):
                # Transpose the mask block through the PE with stride-32 free
                # APs: transpose c yields, for every quarter q, the lhsT mask
                # columns of point class {q*16384 + b*4096 + 32p + c}.
                mf = mf_bufs[b % 2]
                mt = mt_bufs[b]
                # f' = 32p + c: class c picks free elems {32p + c}, stride 32
                mfv = mf.rearrange("r (p c) -> r c p", c=NCLS)
                for h in range(h0, h1):
                    ps4 = ps4_bufs[h % 2]
                    for tt in range(4):
                        c = h * 4 + tt
                        nc.tensor.transpose(ps4[:, tt, :], mfv[:, c, :], ident)
                    nc.scalar.copy(
                        mt[:, h * 512:(h + 1) * 512],
                        ps4.rearrange("p t d -> p (t d)"),
                    )

            k = 0
            jx = 0
            mask_chain(0)
            transposes(0, 0, NCLS // 4)
            for b in range(NBLK):
                mtv = mt_bufs[b].rearrange("p (c m q) -> p c q m", c=NCLS, m=M, q=4)
                for q in range(4):
                    j = q * 4 + b          # x tile covering this block+quarter
                    xb = xb_bufs[jx % NXB]
                    rhs = rhs_bufs[jx % NRHS]
                    jx += 1
                    # Halved x chain (DMA -> ACT convert -> DVE square) to
                    # shorten the per-tile latency before matmuls can start.
                    for h2 in range(2):
                        lo, hi = h2 * (BLK // 2), (h2 + 1) * (BLK // 2)
                        nc.sync.dma_start(
                            out=xb[:, lo:hi], in_=xt[j, :, lo:hi]
                        )
                        # contiguous f32->bf16 convert on ACT (DVE CAST is 1x)
                        nc.scalar.copy(rhs[:, lo:hi], xb[:, lo:hi])
                        # contiguous bf16 square on DVE (2x tensor_tensor)
                        nc.vector.tensor_mul(
                            rhs[:, BLK + lo:BLK + hi],
                            rhs[:, lo:hi], rhs[:, lo:hi],
                        )
                    rhs3 = rhs.rearrange("p (g u) -> p g u", g=2)
                    for c in range(NCLS):
                        nc.tensor.matmul(
                            acc[:, :],
                            lhsT=mtv[:, c, q, :],
                            rhs=rhs3[:, :, c * D:(c + 1) * D],
                            start=(k == 0),
                            stop=(k == n_mm - 1),
                        )
                        k += 1
                    # Software pipeline: next block's mask work interleaves
                    # with this block's matmul groups so the PE never drains
                    # at a block boundary.
                    if b + 1 < NBLK:
                        if q == 0:
                            mask_chain(b + 1)
                        if q >= 2:
                            h0 = (q - 2) * 4
                            transposes(b + 1, h0, h0 + 4)

            outs = singles.tile([M, OUTC], F32, tag="outs")
            nc.vector.tensor_copy(outs, acc)
            nc.sync.dma_start(out=out[:, :], in_=outs)
            nc.sync.dma_start(out=cnts[:, :], in_=cnt_sb)
    # Split multi-wait instructions into EventSemaphore + instruction to
    # satisfy the TRN2 1-wait-per-instruction codegen limit.
    bass_rust.generate_event_semaphores(nc)
    return nc


def _get_nc():
    if "nc" not in _NC_CACHE:
        _NC_CACHE["nc"] = _build_nc()
    return _NC_CACHE["nc"]


def kernel(net_out, pt_offset, mask_embs, mask_pts, logit_scale):
    global LAST
    net_out = np.ascontiguousarray(np.asarray(net_out, dtype=np.float32))
    mask_pts = np.ascontiguousarray(np.asarray(mask_pts, dtype=np.int32))
    mask_embs = np.asarray(mask_embs, dtype=np.float32)

    nc = _get_nc()
    in_maps = [
        {
            "x": net_out[o * P:(o + 1) * P],
            "mask": mask_pts[o].reshape(VIEW_P, VIEW_F),
        }
        for o in range(N_CORES)
    ]
    trace = os.environ.get("KBENCH_TRACE", "0") == "1"
    res = run_bass_kernel_spmd(nc, in_maps, list(range(N_CORES)), trace=trace)
    LAST = res

    accs = np.stack([np.asarray(res.results[o]["out"]) for o in range(N_CORES)])
    mx = accs[:, :, 0:D].astype(np.float64)        # [8, 32, 128]
    sx2 = accs[:, :, D:2 * D].astype(np.float64)   # [8, 32, 128]
    # cnt[m] = sum over quarters q and pieces of the per-row partials
    cnts = np.stack([np.asarray(res.results[o]["cnts"]) for o in range(N_CORES)])
    cnt = cnts.sum(-1).reshape(N_CORES, M, 4).sum(-1)  # [8, 32]

    emb = np.nan_to_num(
        mask_embs.reshape(N_OBJ, M, D).astype(np.float64),
        nan=0.0, posinf=0.0, neginf=0.0,
    )
    t1 = sx2.sum(-1)
    t2 = cnt * (emb * emb).sum(-1)
    t3 = 2.0 * (emb * mx).sum(-1)
    sum_sq = (t1 + t2 - t3).sum()
    total = cnt.sum()
    val = sum_sq / (D * total) if total > 0 else 0.0
    return np.float32(val)
